# revision 31
# baseline (speedup 1.0000x reference)
"""Dissipative Hamiltonian derivation — Trainium2 Bass kernel, 8-core SPMD.

Block-sparse formulation. The pair mask (mvw.T@mvw * m m^T) is nonzero only
for same-molecule pairs: 48 molecules of 23-49 nodes each, so only
sum n_k^2 ~= 51k of the N^2 = 2.36M pairs contribute (46x sparsity).

Math (closed-form gradients, no autodiff):
  vs = sigmoid(v); vq = [vs, q]; R = vq @ W1_w.T; U = R + b
  S[i,j] = ||u_j - r_i||^2 = rn2_i + un2_j - 2 r_i.u_j   (same-mol pairs only)
  dist = softplus(S); T = (dist-2) * dist^-3 * sigmoid(S)
  w_i = mvw[mol(i), i] * m_i
  Praw[a] = sum_i T_ia [w_i r_i | w_i | 0]; Braw[a] = sum_j T_aj [w_j u_j | 0 | w_j]
  -dHdq_a = [2 w_a (PH+BH)_a - 2 w_a u_a Pl_a - 2 w_a r_a Bl_a] @ W1q
  dq = (2/m) softplus(zT) sig(zT) @ W_T[:,64:];  zT = [vs,p] @ W_T.T
  dp = -dHdq + (2/m) softplus(zF) sig(zF) @ (-W_F);  zF = p @ W_F.T
  (the diagonal pair i=i is included on both P and B sides and cancels)

Layout: 6 molecules per core, each padded to a 64-slot. Per core one packed
S tile [128, 192]: partition half h in {0,1} x free slot p in {0,1,2} holds
molecule b = 2p+h (its own rows AND its own columns — column identity differs
per partition half, which is fine since every consumer is per-block).
All-pairs elementwise chain runs ONCE on [128,192]; per-block row sums (B)
and col sums (P) accumulate in one PSUM tile via rhs vectors with the mask
weight folded in (pads have w=0 so they contribute nothing). No collectives:
each core owns whole molecules, so all pair sums are core-local.
Host does the O(N*H) linear precompute and the pad/permute packing;
host packing depends on mvw but the compiled program does not.
"""

import os
import numpy as np

N = 1536
NM = 48
NCORES = 8
MPC = NM // NCORES          # 6 molecules per core
SLOT = 64
NP = 3                      # slot-pairs per core -> 3 row tiles of 128
H = 16
VD = 64
RW = 66                     # rowpack cols: zT16|zF16|mi2|wgt2|u2wn16|r2wn16

_CACHE = {}


def _patch_act_tables():
    """Filter every other ACT table's function set down so Exp/Ln resolve
    uniquely to natural_log_exp_and_others — the insert_act_table_loads
    pass then hoists a single table load instead of thrashing Exp<->Ln."""
    from concourse import bacc as _bacc
    from concourse.hw_specs import get_activation_tables as _orig

    if getattr(_bacc, "_act_tables_patched", False):
        return

    def patched(arch):
        tabs = _orig(arch)
        combined = "natural_log_exp_and_others"
        if combined not in tabs:
            return tabs
        keep = tabs[combined]
        return {
            name: (funcs if name == combined else funcs - keep)
            for name, funcs in tabs.items()
        }

    _bacc.get_activation_tables = patched
    _bacc._act_tables_patched = True


def _build_nc():
    from concourse import bacc, mybir
    import concourse.tile as tile

    STAGE = int(os.environ.get("KSTAGE", "3"))

    _patch_act_tables()

    f32 = mybir.dt.float32
    bf16 = mybir.dt.bfloat16
    AF = mybir.ActivationFunctionType
    ALU = mybir.AluOpType

    nc = bacc.Bacc(None, num_devices=NCORES)

    f32r = mybir.dt.float32r
    # per pair p: [lhsT 128 | rhs 64] with K=36 = two 18-row groups; the
    # lhsT halves are zero-padded block-diagonally so one matmul computes
    # both molecules' S blocks into [128, 64] at PSUM partition offset 0
    # (f32r matmuls reject nonzero out partition offsets)
    su_d = nc.dram_tensor("su", [36, NP * 192], f32, kind="ExternalInput")
    row_d = nc.dram_tensor("rowpk", [128, NP * RW], f32, kind="ExternalInput")
    # bfpk = [P-rhs 54 | identity 128 | zero-diagonal mask 192]
    bf_d = nc.dram_tensor("bfpk", [128, NP * 18 + 128 + NP * SLOT], bf16,
                          kind="ExternalInput")
    up_d = nc.dram_tensor("upk", [SLOT, 2 * NP * 18], bf16, kind="ExternalInput")
    wp_d = nc.dram_tensor("wpk", [H, 96], bf16, kind="ExternalInput")

    dp_d = nc.dram_tensor("dp_s", [NP, 128, 32], f32, kind="ExternalOutput")
    dq_d = nc.dram_tensor("dq_s", [NP, 128, 32], f32, kind="ExternalOutput")

    with tile.TileContext(nc) as tc:
        with (
            tc.tile_pool(name="const", bufs=1) as cp,
            tc.tile_pool(name="work", bufs=2) as wp,
        ):
            # loads in first-need order; rowpack leads (kinetic chain starts
            # on it and its first ACT hoists the one table load)
            row = cp.tile([128, NP * RW], f32, tag="row")
            nc.sync.dma_start(row[:], row_d[:])
            su = cp.tile([36, NP * 192], f32, tag="su")
            nc.scalar.dma_start(su[:], su_d[:])
            bfp = cp.tile([128, NP * 18 + 128 + NP * SLOT], bf16, tag="bfp")
            nc.sync.dma_start(bfp[:], bf_d[:])
            upk = cp.tile([SLOT, 2 * NP * 18], bf16, tag="upk")
            nc.scalar.dma_start(upk[:], up_d[:])
            wpk = cp.tile([H, 96], bf16, tag="wpk")
            nc.sync.dma_start(wpk[:], wp_d[:])
            idb = bfp[:, NP * 18:NP * 18 + 128]
            dmask = bfp[:, NP * 18 + 128:NP * 18 + 128 + NP * SLOT]

            # f32r copy for the S matmuls (PE fp32 path is f32r-only-safe)
            sur = cp.tile([36, NP * 192], f32r, tag="sur")
            nc.vector.tensor_copy(sur[:], su[:])

            with (
                tc.tile_pool(name="psA", bufs=1, space="PSUM") as psA,
                tc.tile_pool(name="psB", bufs=2, space="PSUM") as psB,
                tc.tile_pool(name="psC", bufs=1, space="PSUM") as psC,
                tc.tile_pool(name="psD", bufs=2, space="PSUM") as psD,
                tc.tile_pool(name="psE", bufs=1, space="PSUM") as psE,
            ):
                # ---- kinetic (dq) + dissipated transposes; fills the
                # window while su/bfp/upk stream in ----
                ktss = []
                for p in range(NP):
                    z = row[:, p * RW:p * RW + 32]
                    mi2 = row[:, p * RW + 32:p * RW + 33]
                    et = wp.tile([128, 32], f32, tag="et")
                    nc.scalar.activation(et[:], z, AF.Exp, scale=-1.0)
                    lt = wp.tile([128, 32], f32, tag="lt")
                    nc.scalar.activation(lt[:], et[:], AF.Ln, bias=1.0)
                    sg = wp.tile([128, 32], f32, tag="sg")
                    nc.scalar.activation(sg[:], lt[:], AF.Exp, scale=-1.0)
                    pw = wp.tile([128, 32], f32, tag="pw")
                    nc.vector.tensor_add(pw[:], lt[:], z)
                    gzs = wp.tile([128, 32], bf16, tag="gzs")
                    nc.vector.scalar_tensor_tensor(
                        gzs[:], pw[:], mi2, sg[:], op0=ALU.mult, op1=ALU.mult)
                    ktpT = psB.tile([16, 128], bf16, tag="tr")
                    nc.tensor.transpose(ktpT[:], gzs[:, 0:16], idb)
                    ktsT = wp.tile([16, 128], bf16, tag="ktsT")
                    nc.vector.tensor_copy(ktsT[:], ktpT[:])
                    ktpF = psB.tile([16, 128], bf16, tag="tr")
                    nc.tensor.transpose(ktpF[:], gzs[:, 16:32], idb)
                    ktsF = cp.tile([16, 128], bf16, tag=f"ktsF{p}")
                    nc.vector.tensor_copy(ktsF[:], ktpF[:])
                    ktss.append(ktsF)
                    dqp = psD.tile([128, 32], f32, tag="dq")
                    nc.tensor.matmul(dqp[:], ktsT[:], wpk[:, 0:32],
                                     start=True, stop=True)
                    dqs = wp.tile([128, 32], f32, tag="dqs")
                    nc.vector.tensor_copy(dqs[:], dqp[:])
                    nc.sync.dma_start(dq_d[p], dqs[:])

                if STAGE < 1:
                    for p in range(NP):
                        dpo = wp.tile([128, 32], f32, tag="dpo")
                        nc.vector.tensor_copy(dpo[:], row[:, p * RW:p * RW + 32])
                        nc.sync.dma_start(dp_d[p], dpo[:])

                # ---- pairwise S blocks: 1 K=36 matmul per pair ----
                SP = psA.tile([128, NP * SLOT], f32, tag="sp")
                if STAGE >= 1:
                    for p in range(NP):
                        nc.tensor.matmul(
                            SP[:, 64 * p:64 * p + 64],
                            sur[:, 192 * p:192 * p + 128],
                            sur[:, 192 * p + 128:192 * p + 192],
                            start=True, stop=True)

                # ---- elementwise chain, one pass over [128, 192] ----
                FW = NP * SLOT
                ct = cp.tile([128, FW], bf16, tag="ct")
                if STAGE >= 1:
                    e1 = wp.tile([128, FW], f32, tag="e1")
                    nc.scalar.activation(e1[:], SP[:], AF.Exp, scale=-1.0)
                    l1 = wp.tile([128, FW], f32, tag="l1")
                    nc.scalar.activation(l1[:], e1[:], AF.Ln, bias=1.0)
                    dist = wp.tile([128, FW], f32, tag="dist")
                    nc.vector.tensor_add(dist[:], l1[:], SP[:])
                    lnd = wp.tile([128, FW], f32, tag="lnd")
                    nc.scalar.activation(lnd[:], dist[:], AF.Ln)
                    wts = wp.tile([128, FW], f32, tag="wts")
                    nc.vector.scalar_tensor_tensor(
                        wts[:], lnd[:], 3.0, l1[:], op0=ALU.mult, op1=ALU.add)
                    sp3 = wp.tile([128, FW], f32, tag="sp3")
                    nc.scalar.activation(sp3[:], wts[:], AF.Exp, scale=-1.0)
                    ctr = wp.tile([128, FW], bf16, tag="ctr")
                    nc.vector.scalar_tensor_tensor(
                        ctr[:], dist[:], -2.0, sp3[:], op0=ALU.add, op1=ALU.mult)
                    # zero the block diagonals exactly: the true gradient has
                    # no i==i term, and leaving it in breaks the P/B
                    # cancellation under bf16 rounding (1.5e-2 -> 1.2e-3)
                    nc.gpsimd.tensor_mul(ct[:], ctr[:], dmask)

                if STAGE == 1:
                    for p in range(NP):
                        dpo = wp.tile([128, 32], f32, tag="dpo")
                        nc.vector.tensor_copy(dpo[:], ct[:, 64 * p:64 * p + 32])
                        nc.sync.dma_start(dp_d[p], dpo[:])

                # ---- per-pair: transpose, P+B sums, epilogue ----
                # P and B land in separate column ranges of one PSUM tile:
                # a PE accumulation group whose members use different
                # partition-base operands crashes at runtime
                for p in range(NP if STAGE >= 2 else 0):
                    ttp = psB.tile([64, 128], bf16, tag="tr")
                    nc.tensor.transpose(ttp[:], ct[:, 64 * p:64 * p + 64], idb)
                    tts = wp.tile([64, 128], bf16, tag="tts")
                    nc.vector.tensor_copy(tts[:], ttp[:])
                    acP = psC.tile([128, 18], f32, tag="acP")
                    acB = psC.tile([128, 18], f32, tag="acB")
                    for h in (0, 1):
                        b = 2 * p + h
                        sl_h = slice(64 * h, 64 * h + 64)
                        # P side: col sums over i (native layout)
                        nc.tensor.matmul(
                            acP[sl_h, :], ct[sl_h, 64 * p:64 * p + 64],
                            bfp[sl_h, 18 * p:18 * p + 18],
                            start=True, stop=True)
                        # B side: row sums over j (transposed layout)
                        nc.tensor.matmul(
                            acB[sl_h, :], tts[:, sl_h],
                            upk[:, 18 * b:18 * b + 18],
                            start=True, stop=True)

                    u2wn = row[:, p * RW + 34:p * RW + 50]
                    r2wn = row[:, p * RW + 50:p * RW + 66]
                    wgt2 = row[:, p * RW + 33:p * RW + 34]
                    ac = wp.tile([128, 36], f32, tag="ac")
                    nc.vector.tensor_copy(ac[:, 0:18], acP[:])
                    nc.vector.tensor_copy(ac[:, 18:36], acB[:])
                    if STAGE == 2:
                        dpo = wp.tile([128, 32], f32, tag="dpo")
                        nc.vector.tensor_copy(dpo[:], ac[:, 0:32])
                        nc.sync.dma_start(dp_d[p], dpo[:])
                        continue
                    hsum = wp.tile([128, H], f32, tag="hsum")
                    nc.vector.tensor_add(hsum[:], ac[:, 0:16], ac[:, 18:34])
                    a2 = wp.tile([128, H], f32, tag="a2")
                    nc.vector.tensor_scalar_mul(a2[:], r2wn, ac[:, 35:36])
                    s_ = wp.tile([128, H], f32, tag="s_")
                    nc.vector.scalar_tensor_tensor(
                        s_[:], u2wn, ac[:, 16:17], a2[:],
                        op0=ALU.mult, op1=ALU.add)
                    dn = wp.tile([128, H], bf16, tag="dn")
                    nc.vector.scalar_tensor_tensor(
                        dn[:], hsum[:], wgt2, s_[:],
                        op0=ALU.mult, op1=ALU.add)
                    etp = psB.tile([16, 128], bf16, tag="tr")
                    nc.tensor.transpose(etp[:], dn[:], idb)
                    ets = wp.tile([16, 128], bf16, tag="ets")
                    nc.vector.tensor_copy(ets[:], etp[:])
                    ddp = psE.tile([128, 32], f32, tag="ddp")
                    nc.tensor.matmul(ddp[:], ktss[p][:], wpk[:, 32:64],
                                     start=True, stop=False)
                    nc.tensor.matmul(ddp[:], ets[:], wpk[:, 64:96],
                                     start=False, stop=True)
                    dpo = wp.tile([128, 32], f32, tag="dpo")
                    nc.vector.tensor_copy(dpo[:], ddp[:])
                    nc.sync.dma_start(dp_d[p], dpo[:])

    nc.finalize()
    return nc


def _prepare_in_maps(v, e, m, p, q, mvw, W_T, W1_w, W1_b, W_F):
    import ml_dtypes
    f32 = np.float32
    bf16 = ml_dtypes.bfloat16
    v, m, p, q, mvw = (np.asarray(x, f32) for x in (v, m, p, q, mvw))
    W_T, W1_w, W1_b, W_F = (np.asarray(x, f32) for x in (W_T, W1_w, W1_b, W_F))

    vs = (1.0 / (1.0 + np.exp(-v))).astype(f32)
    vq = np.concatenate([vs, q], axis=1)                      # [N, 96]
    R = (vq @ W1_w.T).astype(f32)                             # [N, 16]
    U = (R + W1_b[None, :]).astype(f32)
    rn2 = np.einsum("nh,nh->n", R, R).astype(f32)
    un2 = np.einsum("nh,nh->n", U, U).astype(f32)
    zT = (np.concatenate([vs, p], axis=1) @ W_T.T).astype(f32)
    zF = (p @ W_F.T).astype(f32)

    mol_id = np.argmax(mvw, axis=0)                           # [N]
    w_node = (mvw[mol_id, np.arange(N)] * m[:, 0]).astype(f32)

    sizes = np.bincount(mol_id, minlength=NM)
    assert sizes.max() <= SLOT, f"molecule of size {sizes.max()} > {SLOT}"
    order = np.argsort(-sizes, kind="stable")
    nodes_of = [np.where(mol_id == k)[0] for k in range(NM)]

    wpk = np.concatenate([W_T[:, VD:], -W_F, W1_w[:, VD:]], axis=1)

    shared = {"wpk": np.ascontiguousarray(wpk.astype(bf16))}
    in_maps = []
    scatter = []    # per core: list of (dram_flat_row, node_idx)
    for c in range(NCORES):
        mols = [order[i] for i in range(c, NM, NCORES)]
        assert len(mols) == MPC
        su = np.zeros((36, NP * 192), f32)
        rowpk = np.zeros((128, NP * RW), f32)
        bfpk = np.zeros((128, NP * 18 + 128 + NP * SLOT), bf16)
        bfpk[:, NP * 18:NP * 18 + 128] = np.eye(128, dtype=bf16)
        # zero-diagonal mask: 1 everywhere except each 64-block's diagonal
        dm = np.ones((128, NP * SLOT), bf16)
        for pp in range(NP):
            for h in (0, 1):
                for t in range(SLOT):
                    dm[64 * h + t, 64 * pp + t] = 0
        bfpk[:, NP * 18 + 128:] = dm
        upk = np.zeros((SLOT, 2 * NP * 18), bf16)
        sc = []
        for b, k in enumerate(mols):
            idx = nodes_of[k]
            n = len(idx)
            pp, h = b // 2, b % 2
            r0 = 64 * h
            # S matmul operands: lhsT at K-rows 18h..18h+18 (block-diagonal
            # zero padding), rhs at the same K-rows
            k0 = 18 * h
            lh0 = 192 * pp + r0               # lhsT col base for this mol
            rh0 = 192 * pp + 128              # rhs col base for this pair
            su[k0:k0 + 16, lh0:lh0 + n] = -2.0 * R[idx].T
            su[k0 + 16, lh0:lh0 + n] = rn2[idx]
            su[k0 + 17, lh0:lh0 + n] = 1.0
            su[k0:k0 + 16, rh0:rh0 + n] = U[idx].T
            su[k0 + 16, rh0:rh0 + n] = 1.0
            su[k0 + 17, rh0:rh0 + n] = un2[idx]
            # row-wise packed data at rows r0:r0+n of pair tile pp
            wn = w_node[idx]
            rowpk[r0:r0 + n, pp * RW + 0:pp * RW + 16] = zT[idx]
            rowpk[r0:r0 + n, pp * RW + 16:pp * RW + 32] = zF[idx]
            rowpk[r0:r0 + n, pp * RW + 32] = 2.0 / m[idx, 0]
            rowpk[r0:r0 + n, pp * RW + 33] = 2.0 * wn
            rowpk[r0:r0 + n, pp * RW + 34:pp * RW + 50] = -2.0 * wn[:, None] * U[idx]
            rowpk[r0:r0 + n, pp * RW + 50:pp * RW + 66] = -2.0 * wn[:, None] * R[idx]
            # P rhs [w r | w | 0] rows at partition r0..
            bfpk[r0:r0 + n, 18 * pp:18 * pp + 16] = wn[:, None] * R[idx]
            bfpk[r0:r0 + n, 18 * pp + 16] = wn
            # B rhs [w u | 0 | w] rows at partitions 0:n
            upk[0:n, 18 * b:18 * b + 16] = wn[:, None] * U[idx]
            upk[0:n, 18 * b + 17] = wn
            for t in range(n):
                sc.append((pp * 128 + r0 + t, idx[t]))
        in_maps.append({
            **shared,
            "su": np.ascontiguousarray(su),
            "rowpk": np.ascontiguousarray(rowpk),
            "bfpk": np.ascontiguousarray(bfpk),
            "upk": np.ascontiguousarray(upk),
        })
        scatter.append(sc)
    return in_maps, scatter


def _ensure_ntff_hook():
    """Make antenv.axon_hooks importable so bass_utils' trace path works."""
    try:
        from antenv.axon_hooks import get_axon_ntff_profile_hook  # noqa: F401
        return True
    except ImportError:
        pass
    import contextlib
    import ctypes
    import sys
    import types

    so_path = "/opt/axon/libaxon_pjrt.so"
    try:
        lib = ctypes.CDLL(so_path)
    except OSError:
        return False
    if not hasattr(lib, "axon_start_nrt_profile"):
        return False
    lib.axon_start_nrt_profile.argtypes = [
        ctypes.POINTER(ctypes.c_int64),
        ctypes.c_size_t,
    ]
    lib.axon_start_nrt_profile.restype = ctypes.c_int64
    lib.axon_stop_nrt_profile.argtypes = [ctypes.c_char_p]
    lib.axon_stop_nrt_profile.restype = ctypes.c_int64

    @contextlib.contextmanager
    def _hook(output_dir, device_ids):
        import jax

        jax.devices()
        if device_ids:
            ids = (ctypes.c_int64 * len(device_ids))(*device_ids)
            rc = lib.axon_start_nrt_profile(ids, len(device_ids))
        else:
            rc = lib.axon_start_nrt_profile(None, 0)
        if rc != 0:
            raise RuntimeError(f"axon_start_nrt_profile rc={rc}")
        try:
            yield
        finally:
            n = lib.axon_stop_nrt_profile(str(output_dir).encode())
            if n < 0:
                raise RuntimeError(f"axon_stop_nrt_profile rc={n}")

    mod = types.ModuleType("antenv.axon_hooks")
    mod.get_axon_ntff_profile_hook = lambda: _hook
    sys.modules["antenv.axon_hooks"] = mod
    try:
        import antenv

        antenv.axon_hooks = mod
    except ImportError:
        pass
    return True


def kernel(v, e, m, p, q, mvw, W_T, W1_w, W1_b, W_F):
    from concourse.bass_utils import run_bass_kernel_spmd

    in_maps, scatter = _prepare_in_maps(v, e, m, p, q, mvw,
                                        W_T, W1_w, W1_b, W_F)

    if "nc" not in _CACHE:
        _CACHE["nc"] = _build_nc()
    nc = _CACHE["nc"]

    trace = bool(os.environ.get("BASS_KERNEL_TRACE")) and _ensure_ntff_hook()
    res = run_bass_kernel_spmd(nc, in_maps, list(range(NCORES)), trace=trace)
    if trace and res.exec_time_ns is not None:
        print(f"HW exec time: {res.exec_time_ns} ns")

    dp = np.zeros((N, 32), np.float32)
    dq = np.zeros((N, 32), np.float32)
    for c in range(NCORES):
        dps = res.results[c]["dp_s"].reshape(NP * 128, 32)
        dqs = res.results[c]["dq_s"].reshape(NP * 128, 32)
        rows = np.array([r for r, _ in scatter[c]])
        nodes = np.array([nidx for _, nidx in scatter[c]])
        dp[nodes] = dps[rows]
        dq[nodes] = dqs[rows]
    return dp, dq


# revision 32
# speedup vs baseline: 1.0207x; 1.0207x over previous
"""Dissipative Hamiltonian derivation — Trainium2 Bass kernel, 8-core SPMD.

Block-sparse formulation. The pair mask (mvw.T@mvw * m m^T) is nonzero only
for same-molecule pairs: 48 molecules of 23-49 nodes each, so only
sum n_k^2 ~= 51k of the N^2 = 2.36M pairs contribute (46x sparsity).

Math (closed-form gradients, no autodiff):
  vs = sigmoid(v); vq = [vs, q]; R = vq @ W1_w.T; U = R + b
  S[i,j] = ||u_j - r_i||^2 = rn2_i + un2_j - 2 r_i.u_j   (same-mol pairs only)
  dist = softplus(S); T = (dist-2) * dist^-3 * sigmoid(S), diag zeroed
  w_i = mvw[mol(i), i] * m_i
  Praw[a] = sum_i T_ia [w_i r_i | w_i]; Braw[a] = sum_j T_aj [w_j u_j | w_j]
  dp_a = [2 w_a (PH+BH)_a - 2 w_a u_a Pl_a - 2 w_a r_a Bl_a] @ W1q
         - (2/m) softplus(zF) sig(zF) @ W_F
  dq = (2/m) softplus(zT) sig(zT) @ W_T[:,64:]

Layout: 6 molecules per core, 64-padded. One packed S tile [128, 192]:
partition half h x free slot p holds molecule 2p+h (its own rows AND cols).
The whole S tile comes from ONE K=108 f32r matmul: lhsT/rhs are written
with 36-row K-groups per pair, zero elsewhere, so cross-pair terms vanish.
Elementwise chain runs once on [128,192]; per-block row sums (B, via a PE
transpose of C) and col sums (P) go to separate PSUM tiles (a PE
accumulation group whose members use different partition bases crashes HW).
Transposed 16-row operands are batched into 32-aligned slots so one PE
transpose serves 3 pairs (matmul lhsT/rhs base partitions must match and
be 0/32/64 — weights are host-replicated at all three bases).
No collectives: each core owns whole molecules. Host does the O(N*H)
linear precompute and packing; the compiled program is input-independent.
"""

import os
import numpy as np

N = 1536
NM = 48
NCORES = 8
MPC = NM // NCORES          # 6 molecules per core
SLOT = 64
NP = 3                      # slot-pairs per core -> 3 row tiles of 128
H = 16
VD = 64
FW = NP * SLOT              # 192
RW = 198                    # rowpack: z_all 96 | mi2 3 | wgt2 3 | u2wn 48 | r2wn 48
BFW = NP * 18 + 128 + FW    # bfpk: P-rhs 54 | identity 128 | dmask 192

_CACHE = {}


def _patch_act_tables():
    """Filter every other ACT table's function set down so Exp/Ln resolve
    uniquely to natural_log_exp_and_others — the insert_act_table_loads
    pass then hoists a single table load instead of thrashing Exp<->Ln."""
    from concourse import bacc as _bacc
    from concourse.hw_specs import get_activation_tables as _orig

    if getattr(_bacc, "_act_tables_patched", False):
        return

    def patched(arch):
        tabs = _orig(arch)
        combined = "natural_log_exp_and_others"
        if combined not in tabs:
            return tabs
        keep = tabs[combined]
        return {
            name: (funcs if name == combined else funcs - keep)
            for name, funcs in tabs.items()
        }

    _bacc.get_activation_tables = patched
    _bacc._act_tables_patched = True


def _build_nc():
    from concourse import bacc, mybir
    import concourse.tile as tile

    _patch_act_tables()

    f32 = mybir.dt.float32
    f32r = mybir.dt.float32r
    bf16 = mybir.dt.bfloat16
    AF = mybir.ActivationFunctionType
    ALU = mybir.AluOpType

    nc = bacc.Bacc(None, num_devices=NCORES)

    su_d = nc.dram_tensor("su", [108, 320], f32, kind="ExternalInput")
    row_d = nc.dram_tensor("rowpk", [128, RW], f32, kind="ExternalInput")
    bf_d = nc.dram_tensor("bfpk", [128, BFW], bf16, kind="ExternalInput")
    up_d = nc.dram_tensor("upk", [SLOT, 2 * NP * 18], bf16, kind="ExternalInput")
    wp_d = nc.dram_tensor("wpk", [80, 96], bf16, kind="ExternalInput")

    dp_d = nc.dram_tensor("dp_s", [NP, 128, 32], f32, kind="ExternalOutput")
    dq_d = nc.dram_tensor("dq_s", [NP, 128, 32], f32, kind="ExternalOutput")

    with tile.TileContext(nc) as tc:
        with (
            tc.tile_pool(name="const", bufs=1) as cp,
            tc.tile_pool(name="work", bufs=2) as wp,
        ):
            # 16-row transpose operands live in 32-aligned slots; the pad
            # slots are transposed as garbage but never read — memset once
            # so nothing reads uninitialized SBUF
            gza = cp.tile([128, 192], bf16, tag="gza")
            nc.vector.memset(gza[:], 0.0)
            dna = cp.tile([128, 96], bf16, tag="dna")
            nc.vector.memset(dna[:], 0.0)

            # input DMAs: sync queue for the critical path, gpsimd (SWDGE)
            # for late-need data; scalar queue stays free for ACT work
            row = cp.tile([128, RW], f32, tag="row")
            nc.sync.dma_start(row[:, 0:99], row_d[:, 0:99])
            su = cp.tile([108, 320], f32, tag="su")
            nc.sync.dma_start(su[:], su_d[:])
            bfp = cp.tile([128, BFW], bf16, tag="bfp")
            nc.sync.dma_start(bfp[:], bf_d[:])
            nc.gpsimd.dma_start(row[:, 99:RW], row_d[:, 99:RW])
            upk = cp.tile([SLOT, 2 * NP * 18], bf16, tag="upk")
            nc.gpsimd.dma_start(upk[:], up_d[:])
            wpk = cp.tile([80, 96], bf16, tag="wpk")
            nc.gpsimd.dma_start(wpk[:], wp_d[:])
            idb = bfp[:, NP * 18:NP * 18 + 128]
            dmask = bfp[:, NP * 18 + 128:BFW]

            sur = cp.tile([108, 320], f32r, tag="sur")
            nc.vector.tensor_copy(sur[:], su[:])

            with (
                tc.tile_pool(name="ps1", bufs=1, space="PSUM") as ps1,
                tc.tile_pool(name="ps2", bufs=2, space="PSUM") as ps2,
            ):
                # ---- pairwise: ONE K=108 S matmul, elementwise once ----
                SP = ps1.tile([128, FW], f32, tag="sp")
                nc.tensor.matmul(SP[:], sur[:, 0:128], sur[:, 128:320],
                                 start=True, stop=True)
                e1 = wp.tile([128, FW], f32, tag="e1")
                nc.scalar.activation(e1[:], SP[:], AF.Exp, scale=-1.0)
                l1 = wp.tile([128, FW], f32, tag="l1")
                nc.scalar.activation(l1[:], e1[:], AF.Ln, bias=1.0)
                dist = wp.tile([128, FW], f32, tag="dist")
                nc.vector.tensor_add(dist[:], l1[:], SP[:])
                lnd = wp.tile([128, FW], f32, tag="lnd")
                nc.scalar.activation(lnd[:], dist[:], AF.Ln)
                wts = wp.tile([128, FW], f32, tag="wts")
                nc.vector.scalar_tensor_tensor(
                    wts[:], lnd[:], 3.0, l1[:], op0=ALU.mult, op1=ALU.add)
                sp3 = wp.tile([128, FW], f32, tag="sp3")
                nc.scalar.activation(sp3[:], wts[:], AF.Exp, scale=-1.0)
                ctr = wp.tile([128, FW], bf16, tag="ctr")
                nc.vector.scalar_tensor_tensor(
                    ctr[:], dist[:], -2.0, sp3[:], op0=ALU.add, op1=ALU.mult)
                # zero the block diagonals exactly: the true gradient has no
                # i==i term, and leaving it in breaks the P/B cancellation
                # under bf16 rounding (1.5e-2 -> 1.2e-3 rel err)
                ct = cp.tile([128, FW], bf16, tag="ct")
                nc.gpsimd.tensor_mul(ct[:], ctr[:], dmask)

                # ---- kinetic/dissipated, batched over the 3 row tiles ----
                za = row[:, 0:96]
                et = wp.tile([128, 96], f32, tag="et")
                nc.scalar.activation(et[:], za, AF.Exp, scale=-1.0)
                lt = wp.tile([128, 96], f32, tag="lt")
                nc.scalar.activation(lt[:], et[:], AF.Ln, bias=1.0)
                sg = wp.tile([128, 96], f32, tag="sg")
                nc.scalar.activation(sg[:], lt[:], AF.Exp, scale=-1.0)
                pw = wp.tile([128, 96], f32, tag="pw")
                nc.vector.tensor_add(pw[:], lt[:], za)
                for p in range(NP):
                    mi2 = row[:, 96 + p:97 + p]
                    sT = slice(32 * p, 32 * p + 16)
                    sF = slice(32 * p + 16, 32 * p + 32)
                    nc.vector.scalar_tensor_tensor(
                        gza[:, 32 * p:32 * p + 16], pw[:, sT], mi2, sg[:, sT],
                        op0=ALU.mult, op1=ALU.mult)
                    nc.vector.scalar_tensor_tensor(
                        gza[:, 96 + 32 * p:96 + 32 * p + 16], pw[:, sF], mi2,
                        sg[:, sF], op0=ALU.mult, op1=ALU.mult)
                trT = ps2.tile([96, 128], bf16, tag="tr")
                nc.tensor.transpose(trT[:], gza[:, 0:96], idb)
                gzT = cp.tile([96, 128], bf16, tag="gzT")
                nc.vector.tensor_copy(gzT[:], trT[:])
                trF = ps2.tile([96, 128], bf16, tag="tr")
                nc.tensor.transpose(trF[:], gza[:, 96:192], idb)
                gzF = cp.tile([96, 128], bf16, tag="gzF")
                nc.vector.tensor_copy(gzF[:], trF[:])
                for p in range(NP):
                    s3 = slice(32 * p, 32 * p + 16)
                    dqp = ps1.tile([128, 32], f32, tag="dq")
                    nc.tensor.matmul(dqp[:], gzT[s3, :], wpk[s3, 0:32],
                                     start=True, stop=True)
                    dqs = wp.tile([128, 32], f32, tag="dqs")
                    nc.vector.tensor_copy(dqs[:], dqp[:])
                    nc.sync.dma_start(dq_d[p], dqs[:])

                # ---- per-pair: transpose, P+B sums, epilogue to dn ----
                for p in range(NP):
                    ttp = ps2.tile([96, 128], bf16, tag="tr")
                    nc.tensor.transpose(ttp[0:64, :],
                                        ct[:, 64 * p:64 * p + 64], idb)
                    tts = wp.tile([64, 128], bf16, tag="tts")
                    nc.vector.tensor_copy(tts[:], ttp[0:64, :])
                    acP = ps1.tile([128, 18], f32, tag="acP")
                    acB = ps1.tile([128, 18], f32, tag="acB")
                    for h in (0, 1):
                        b = 2 * p + h
                        sl_h = slice(64 * h, 64 * h + 64)
                        # P side: col sums over i (native layout)
                        nc.tensor.matmul(
                            acP[sl_h, :], ct[sl_h, 64 * p:64 * p + 64],
                            bfp[sl_h, 18 * p:18 * p + 18],
                            start=True, stop=True)
                        # B side: row sums over j (transposed layout)
                        nc.tensor.matmul(
                            acB[sl_h, :], tts[:, sl_h],
                            upk[:, 18 * b:18 * b + 18],
                            start=True, stop=True)
                    u2wn = row[:, 102 + 16 * p:102 + 16 * p + 16]
                    r2wn = row[:, 150 + 16 * p:150 + 16 * p + 16]
                    wgt2 = row[:, 99 + p:100 + p]
                    ac = wp.tile([128, 36], f32, tag="ac")
                    nc.vector.tensor_copy(ac[:, 0:18], acP[:])
                    nc.vector.tensor_copy(ac[:, 18:36], acB[:])
                    hsum = wp.tile([128, H], f32, tag="hsum")
                    nc.vector.tensor_add(hsum[:], ac[:, 0:16], ac[:, 18:34])
                    a2 = wp.tile([128, H], f32, tag="a2")
                    nc.vector.tensor_scalar_mul(a2[:], r2wn, ac[:, 35:36])
                    s_ = wp.tile([128, H], f32, tag="s_")
                    nc.vector.scalar_tensor_tensor(
                        s_[:], u2wn, ac[:, 16:17], a2[:],
                        op0=ALU.mult, op1=ALU.add)
                    nc.vector.scalar_tensor_tensor(
                        dna[:, 32 * p:32 * p + 16], hsum[:], wgt2, s_[:],
                        op0=ALU.mult, op1=ALU.add)

                # ---- batched dn transpose, then dp per pair ----
                etp = ps2.tile([96, 128], bf16, tag="tr")
                nc.tensor.transpose(etp[:], dna[:], idb)
                ets = cp.tile([96, 128], bf16, tag="ets")
                nc.vector.tensor_copy(ets[:], etp[:])
                for p in range(NP):
                    s3 = slice(32 * p, 32 * p + 16)
                    ddp = ps2.tile([128, 32], f32, tag="ddp")
                    nc.tensor.matmul(ddp[:], gzF[s3, :], wpk[s3, 32:64],
                                     start=True, stop=False)
                    nc.tensor.matmul(ddp[:], ets[s3, :], wpk[s3, 64:96],
                                     start=False, stop=True)
                    dpo = wp.tile([128, 32], f32, tag="dpo")
                    nc.vector.tensor_copy(dpo[:], ddp[:])
                    nc.sync.dma_start(dp_d[p], dpo[:])

    nc.finalize()
    return nc


def _prepare_in_maps(v, e, m, p, q, mvw, W_T, W1_w, W1_b, W_F):
    import ml_dtypes
    f32 = np.float32
    bf16 = ml_dtypes.bfloat16
    v, m, p, q, mvw = (np.asarray(x, f32) for x in (v, m, p, q, mvw))
    W_T, W1_w, W1_b, W_F = (np.asarray(x, f32) for x in (W_T, W1_w, W1_b, W_F))

    vs = (1.0 / (1.0 + np.exp(-v))).astype(f32)
    vq = np.concatenate([vs, q], axis=1)                      # [N, 96]
    R = (vq @ W1_w.T).astype(f32)                             # [N, 16]
    U = (R + W1_b[None, :]).astype(f32)
    rn2 = np.einsum("nh,nh->n", R, R).astype(f32)
    un2 = np.einsum("nh,nh->n", U, U).astype(f32)
    zT = (np.concatenate([vs, p], axis=1) @ W_T.T).astype(f32)
    zF = (p @ W_F.T).astype(f32)

    mol_id = np.argmax(mvw, axis=0)                           # [N]
    w_node = (mvw[mol_id, np.arange(N)] * m[:, 0]).astype(f32)

    sizes = np.bincount(mol_id, minlength=NM)
    assert sizes.max() <= SLOT, f"molecule of size {sizes.max()} > {SLOT}"
    order = np.argsort(-sizes, kind="stable")
    nodes_of = [np.where(mol_id == k)[0] for k in range(NM)]

    # weights replicated at partition bases 0/32/64 (matmul lhsT/rhs bases
    # must match); cols: [WTp | -WF | W1q]
    wkb = np.concatenate([W_T[:, VD:], -W_F, W1_w[:, VD:]], axis=1)
    wpk = np.zeros((80, 96), f32)
    for b0 in (0, 32, 64):
        wpk[b0:b0 + H, :] = wkb

    shared = {"wpk": np.ascontiguousarray(wpk.astype(bf16))}
    in_maps = []
    scatter = []    # per core: (dram_flat_row, node_idx) pairs
    for c in range(NCORES):
        mols = [order[i] for i in range(c, NM, NCORES)]
        su = np.zeros((108, 320), f32)
        rowpk = np.zeros((128, RW), f32)
        bfpk = np.zeros((128, BFW), bf16)
        bfpk[:, NP * 18:NP * 18 + 128] = np.eye(128, dtype=bf16)
        dm = np.ones((128, FW), bf16)
        for pp in range(NP):
            for t in range(SLOT):
                dm[t, 64 * pp + t] = 0
                dm[64 + t, 64 * pp + t] = 0
        bfpk[:, NP * 18 + 128:] = dm
        upk = np.zeros((SLOT, 2 * NP * 18), bf16)
        sc = []
        for b, k in enumerate(mols):
            idx = nodes_of[k]
            n = len(idx)
            pp, h = b // 2, b % 2
            r0 = 64 * h
            # S matmul: K-group rows 36*pp + 18*h
            k0 = 36 * pp + 18 * h
            su[k0:k0 + 16, r0:r0 + n] = -2.0 * R[idx].T
            su[k0 + 16, r0:r0 + n] = rn2[idx]
            su[k0 + 17, r0:r0 + n] = 1.0
            su[k0:k0 + 16, 128 + 64 * pp:128 + 64 * pp + n] = U[idx].T
            su[k0 + 16, 128 + 64 * pp:128 + 64 * pp + n] = 1.0
            su[k0 + 17, 128 + 64 * pp:128 + 64 * pp + n] = un2[idx]
            # row-wise packed data at rows r0:r0+n of pair tile pp
            wn = w_node[idx]
            rowpk[r0:r0 + n, 32 * pp:32 * pp + 16] = zT[idx]
            rowpk[r0:r0 + n, 32 * pp + 16:32 * pp + 32] = zF[idx]
            rowpk[r0:r0 + n, 96 + pp] = 2.0 / m[idx, 0]
            rowpk[r0:r0 + n, 99 + pp] = 2.0 * wn
            rowpk[r0:r0 + n, 102 + 16 * pp:118 + 16 * pp] = \
                -2.0 * wn[:, None] * U[idx]
            rowpk[r0:r0 + n, 150 + 16 * pp:166 + 16 * pp] = \
                -2.0 * wn[:, None] * R[idx]
            # P rhs [w r | w | 0] rows at partition r0..
            bfpk[r0:r0 + n, 18 * pp:18 * pp + 16] = wn[:, None] * R[idx]
            bfpk[r0:r0 + n, 18 * pp + 16] = wn
            # B rhs [w u | 0 | w] rows at partitions 0:n
            upk[0:n, 18 * b:18 * b + 16] = wn[:, None] * U[idx]
            upk[0:n, 18 * b + 17] = wn
            for t in range(n):
                sc.append((pp * 128 + r0 + t, idx[t]))
        in_maps.append({
            **shared,
            "su": np.ascontiguousarray(su),
            "rowpk": np.ascontiguousarray(rowpk),
            "bfpk": np.ascontiguousarray(bfpk),
            "upk": np.ascontiguousarray(upk),
        })
        scatter.append(sc)
    return in_maps, scatter


def _ensure_ntff_hook():
    """Make antenv.axon_hooks importable so bass_utils' trace path works."""
    try:
        from antenv.axon_hooks import get_axon_ntff_profile_hook  # noqa: F401
        return True
    except ImportError:
        pass
    import contextlib
    import ctypes
    import sys
    import types

    so_path = "/opt/axon/libaxon_pjrt.so"
    try:
        lib = ctypes.CDLL(so_path)
    except OSError:
        return False
    if not hasattr(lib, "axon_start_nrt_profile"):
        return False
    lib.axon_start_nrt_profile.argtypes = [
        ctypes.POINTER(ctypes.c_int64),
        ctypes.c_size_t,
    ]
    lib.axon_start_nrt_profile.restype = ctypes.c_int64
    lib.axon_stop_nrt_profile.argtypes = [ctypes.c_char_p]
    lib.axon_stop_nrt_profile.restype = ctypes.c_int64

    @contextlib.contextmanager
    def _hook(output_dir, device_ids):
        import jax

        jax.devices()
        if device_ids:
            ids = (ctypes.c_int64 * len(device_ids))(*device_ids)
            rc = lib.axon_start_nrt_profile(ids, len(device_ids))
        else:
            rc = lib.axon_start_nrt_profile(None, 0)
        if rc != 0:
            raise RuntimeError(f"axon_start_nrt_profile rc={rc}")
        try:
            yield
        finally:
            n = lib.axon_stop_nrt_profile(str(output_dir).encode())
            if n < 0:
                raise RuntimeError(f"axon_stop_nrt_profile rc={n}")

    mod = types.ModuleType("antenv.axon_hooks")
    mod.get_axon_ntff_profile_hook = lambda: _hook
    sys.modules["antenv.axon_hooks"] = mod
    try:
        import antenv

        antenv.axon_hooks = mod
    except ImportError:
        pass
    return True


def kernel(v, e, m, p, q, mvw, W_T, W1_w, W1_b, W_F):
    from concourse.bass_utils import run_bass_kernel_spmd

    in_maps, scatter = _prepare_in_maps(v, e, m, p, q, mvw,
                                        W_T, W1_w, W1_b, W_F)

    if "nc" not in _CACHE:
        _CACHE["nc"] = _build_nc()
    nc = _CACHE["nc"]

    trace = bool(os.environ.get("BASS_KERNEL_TRACE")) and _ensure_ntff_hook()
    res = run_bass_kernel_spmd(nc, in_maps, list(range(NCORES)), trace=trace)
    if trace and res.exec_time_ns is not None:
        print(f"HW exec time: {res.exec_time_ns} ns")

    dp = np.zeros((N, 32), np.float32)
    dq = np.zeros((N, 32), np.float32)
    for c in range(NCORES):
        dps = res.results[c]["dp_s"].reshape(NP * 128, 32)
        dqs = res.results[c]["dq_s"].reshape(NP * 128, 32)
        rows = np.array([r for r, _ in scatter[c]])
        nodes = np.array([nidx for _, nidx in scatter[c]])
        dp[nodes] = dps[rows]
        dq[nodes] = dqs[rows]
    return dp, dq


# revision 33
# speedup vs baseline: 1.0711x; 1.0494x over previous
"""Dissipative Hamiltonian derivation — Trainium2 Bass kernel, 8-core SPMD.

Block-sparse formulation. The pair mask (mvw.T@mvw * m m^T) is nonzero only
for same-molecule pairs: 48 molecules of 23-49 nodes each, so only
sum n_k^2 ~= 51k of the N^2 = 2.36M pairs contribute (46x sparsity).

Math (closed-form gradients, no autodiff):
  vs = sigmoid(v); vq = [vs, q]; R = vq @ W1_w.T; U = R + b
  S[i,j] = ||u_j - r_i||^2 = rn2_i + un2_j - 2 r_i.u_j   (same-mol pairs only)
  dist = softplus(S); T = (dist-2) * dist^-3 * sigmoid(S), diag zeroed
  w_i = mvw[mol(i), i] * m_i
  Praw[a] = sum_i T_ia [w_i r_i | w_i]; Braw[a] = sum_j T_aj [w_j u_j | w_j]
  dp_a = [2 w_a (PH+BH)_a - 2 w_a u_a Pl_a - 2 w_a r_a Bl_a] @ W1q
         - (2/m) softplus(zF) sig(zF) @ W_F
  dq = (2/m) softplus(zT) sig(zT) @ W_T[:,64:]

Layout: 6 molecules per core, 64-padded. One packed S tile [128, 192]:
partition half h x free slot p holds molecule 2p+h (its own rows AND cols).
The whole S tile comes from ONE K=108 f32r matmul: lhsT/rhs are written
with 36-row K-groups per pair, zero elsewhere, so cross-pair terms vanish.
Elementwise chain runs once on [128,192]; per-block row sums (B, via a PE
transpose of C) and col sums (P) go to separate PSUM tiles (a PE
accumulation group whose members use different partition bases crashes HW).
Transposed 16-row operands are batched into 32-aligned slots so one PE
transpose serves 3 pairs (matmul lhsT/rhs base partitions must match and
be 0/32/64 — weights are host-replicated at all three bases).
No collectives: each core owns whole molecules. Host does the O(N*H)
linear precompute and packing; the compiled program is input-independent.
"""

import os
import numpy as np

N = 1536
NM = 48
NCORES = 8
MPC = NM // NCORES          # 6 molecules per core
SLOT = 64
NP = 3                      # slot-pairs per core -> 3 row tiles of 128
H = 16
VD = 64
FW = NP * SLOT              # 192
RW = 198                    # rowpack: z_all 96 | mi2 3 | wgt2 3 | u2wn 48 | r2wn 48
BFW = NP * 18 + 128 + FW    # bfpk: P-rhs 54 | identity 128 | dmask 192

_CACHE = {}


def _patch_act_tables():
    """Filter every other ACT table's function set down so Exp/Ln resolve
    uniquely to natural_log_exp_and_others — the insert_act_table_loads
    pass then hoists a single table load instead of thrashing Exp<->Ln."""
    from concourse import bacc as _bacc
    from concourse.hw_specs import get_activation_tables as _orig

    if getattr(_bacc, "_act_tables_patched", False):
        return

    def patched(arch):
        tabs = _orig(arch)
        combined = "natural_log_exp_and_others"
        if combined not in tabs:
            return tabs
        keep = tabs[combined]
        return {
            name: (funcs if name == combined else funcs - keep)
            for name, funcs in tabs.items()
        }

    _bacc.get_activation_tables = patched
    _bacc._act_tables_patched = True


def _build_nc():
    from concourse import bacc, mybir
    import concourse.tile as tile

    _patch_act_tables()

    f32 = mybir.dt.float32
    f32r = mybir.dt.float32r
    bf16 = mybir.dt.bfloat16
    AF = mybir.ActivationFunctionType
    ALU = mybir.AluOpType

    nc = bacc.Bacc(None, num_devices=NCORES)

    su_d = nc.dram_tensor("su", [108, 320], f32, kind="ExternalInput")
    row_d = nc.dram_tensor("rowpk", [128, RW], f32, kind="ExternalInput")
    bf_d = nc.dram_tensor("bfpk", [128, BFW], bf16, kind="ExternalInput")
    up_d = nc.dram_tensor("upk", [SLOT, 2 * NP * 18], bf16, kind="ExternalInput")
    wp_d = nc.dram_tensor("wpk", [80, 96], bf16, kind="ExternalInput")

    dp_d = nc.dram_tensor("dp_s", [NP, 128, 32], f32, kind="ExternalOutput")
    dq_d = nc.dram_tensor("dq_s", [NP, 128, 32], f32, kind="ExternalOutput")

    with tile.TileContext(nc) as tc:
        with (
            tc.tile_pool(name="const", bufs=1) as cp,
            tc.tile_pool(name="work", bufs=2) as wp,
            tc.tile_pool(name="out3", bufs=3) as op3,
        ):
            # 16-row transpose operands live in 32-aligned slots; the pad
            # slots are transposed as garbage but never read — memset once
            # so nothing reads uninitialized SBUF
            gza = cp.tile([128, 192], bf16, tag="gza")
            nc.vector.memset(gza[:], 0.0)
            dna = cp.tile([128, 96], bf16, tag="dna")
            nc.vector.memset(dna[:], 0.0)

            # input DMAs: sync queue for the critical path, gpsimd (SWDGE)
            # for late-need data; scalar queue stays free for ACT work
            su = cp.tile([108, 320], f32, tag="su")
            nc.sync.dma_start(su[:], su_d[:])
            row = cp.tile([128, RW], f32, tag="row")
            nc.sync.dma_start(row[:, 0:99], row_d[:, 0:99])
            bfp = cp.tile([128, BFW], bf16, tag="bfp")
            nc.sync.dma_start(bfp[:], bf_d[:])
            nc.gpsimd.dma_start(row[:, 99:RW], row_d[:, 99:RW])
            upk = cp.tile([SLOT, 2 * NP * 18], bf16, tag="upk")
            nc.gpsimd.dma_start(upk[:], up_d[:])
            wpk = cp.tile([80, 96], bf16, tag="wpk")
            nc.gpsimd.dma_start(wpk[:], wp_d[:])
            idb = bfp[:, NP * 18:NP * 18 + 128]
            dmask = bfp[:, NP * 18 + 128:BFW]

            sur = cp.tile([108, 320], f32r, tag="sur")
            nc.vector.tensor_copy(sur[:], su[:])

            with (
                tc.tile_pool(name="ps1", bufs=1, space="PSUM") as ps1,
                tc.tile_pool(name="ps2", bufs=2, space="PSUM") as ps2,
            ):
                # ---- pairwise: ONE K=108 S matmul, elementwise once ----
                SP = ps1.tile([128, FW], f32, tag="sp")
                nc.tensor.matmul(SP[:], sur[:, 0:128], sur[:, 128:320],
                                 start=True, stop=True)
                e1 = wp.tile([128, FW], f32, tag="e1")
                nc.scalar.activation(e1[:], SP[:], AF.Exp, scale=-1.0)
                l1 = wp.tile([128, FW], f32, tag="l1")
                nc.scalar.activation(l1[:], e1[:], AF.Ln, bias=1.0)
                dist = wp.tile([128, FW], f32, tag="dist")
                nc.vector.tensor_add(dist[:], l1[:], SP[:])
                lnd = wp.tile([128, FW], f32, tag="lnd")
                nc.scalar.activation(lnd[:], dist[:], AF.Ln)
                wts = wp.tile([128, FW], f32, tag="wts")
                nc.vector.scalar_tensor_tensor(
                    wts[:], lnd[:], 3.0, l1[:], op0=ALU.mult, op1=ALU.add)
                sp3 = wp.tile([128, FW], f32, tag="sp3")
                nc.scalar.activation(sp3[:], wts[:], AF.Exp, scale=-1.0)
                ctr = wp.tile([128, FW], bf16, tag="ctr")
                nc.vector.scalar_tensor_tensor(
                    ctr[:], dist[:], -2.0, sp3[:], op0=ALU.add, op1=ALU.mult)
                # zero the block diagonals exactly: the true gradient has no
                # i==i term, and leaving it in breaks the P/B cancellation
                # under bf16 rounding (1.5e-2 -> 1.2e-3 rel err)
                ct = cp.tile([128, FW], bf16, tag="ct")
                nc.vector.tensor_mul(ct[:], ctr[:], dmask)

                # ---- kinetic/dissipated, batched over the 3 row tiles ----
                za = row[:, 0:96]
                et = wp.tile([128, 96], f32, tag="et")
                nc.scalar.activation(et[:], za, AF.Exp, scale=-1.0)
                lt = wp.tile([128, 96], f32, tag="lt")
                nc.scalar.activation(lt[:], et[:], AF.Ln, bias=1.0)
                sg = wp.tile([128, 96], f32, tag="sg")
                nc.scalar.activation(sg[:], lt[:], AF.Exp, scale=-1.0)
                pw = wp.tile([128, 96], f32, tag="pw")
                nc.vector.tensor_add(pw[:], lt[:], za)
                for p in range(NP):
                    mi2 = row[:, 96 + p:97 + p]
                    sT = slice(32 * p, 32 * p + 16)
                    sF = slice(32 * p + 16, 32 * p + 32)
                    nc.vector.scalar_tensor_tensor(
                        gza[:, 32 * p:32 * p + 16], pw[:, sT], mi2, sg[:, sT],
                        op0=ALU.mult, op1=ALU.mult)
                    nc.vector.scalar_tensor_tensor(
                        gza[:, 96 + 32 * p:96 + 32 * p + 16], pw[:, sF], mi2,
                        sg[:, sF], op0=ALU.mult, op1=ALU.mult)
                trT = ps2.tile([96, 128], bf16, tag="tr")
                nc.tensor.transpose(trT[:], gza[:, 0:96], idb)
                gzT = cp.tile([96, 128], bf16, tag="gzT")
                nc.vector.tensor_copy(gzT[:], trT[:])
                trF = ps2.tile([96, 128], bf16, tag="tr")
                nc.tensor.transpose(trF[:], gza[:, 96:192], idb)
                gzF = cp.tile([96, 128], bf16, tag="gzF")
                nc.vector.tensor_copy(gzF[:], trF[:])
                for p in range(NP):
                    s3 = slice(32 * p, 32 * p + 16)
                    dqp = ps1.tile([128, 32], f32, tag="dq")
                    nc.tensor.matmul(dqp[:], gzT[s3, :], wpk[s3, 0:32],
                                     start=True, stop=True)
                    dqs = op3.tile([128, 32], f32, tag="dqs")
                    nc.vector.tensor_copy(dqs[:], dqp[:])
                    nc.sync.dma_start(dq_d[p], dqs[:])

                # ---- per-pair: transpose, P+B sums, epilogue to dn ----
                for p in range(NP):
                    ttp = ps2.tile([96, 128], bf16, tag="tr")
                    nc.tensor.transpose(ttp[0:64, :],
                                        ct[:, 64 * p:64 * p + 64], idb)
                    tts = wp.tile([64, 128], bf16, tag="tts")
                    nc.vector.tensor_copy(tts[:], ttp[0:64, :])
                    acP = ps1.tile([128, 18], f32, tag="acP")
                    acB = ps1.tile([128, 18], f32, tag="acB")
                    for h in (0, 1):
                        b = 2 * p + h
                        sl_h = slice(64 * h, 64 * h + 64)
                        # P side: col sums over i (native layout)
                        nc.tensor.matmul(
                            acP[sl_h, :], ct[sl_h, 64 * p:64 * p + 64],
                            bfp[sl_h, 18 * p:18 * p + 18],
                            start=True, stop=True)
                        # B side: row sums over j (transposed layout)
                        nc.tensor.matmul(
                            acB[sl_h, :], tts[:, sl_h],
                            upk[:, 18 * b:18 * b + 18],
                            start=True, stop=True)
                    u2wn = row[:, 102 + 16 * p:102 + 16 * p + 16]
                    r2wn = row[:, 150 + 16 * p:150 + 16 * p + 16]
                    wgt2 = row[:, 99 + p:100 + p]
                    ac = wp.tile([128, 36], f32, tag="ac")
                    nc.vector.tensor_copy(ac[:, 0:18], acP[:])
                    nc.vector.tensor_copy(ac[:, 18:36], acB[:])
                    hsum = wp.tile([128, H], f32, tag="hsum")
                    nc.vector.tensor_add(hsum[:], ac[:, 0:16], ac[:, 18:34])
                    a2 = wp.tile([128, H], f32, tag="a2")
                    nc.vector.tensor_scalar_mul(a2[:], r2wn, ac[:, 35:36])
                    s_ = wp.tile([128, H], f32, tag="s_")
                    nc.vector.scalar_tensor_tensor(
                        s_[:], u2wn, ac[:, 16:17], a2[:],
                        op0=ALU.mult, op1=ALU.add)
                    nc.vector.scalar_tensor_tensor(
                        dna[:, 32 * p:32 * p + 16], hsum[:], wgt2, s_[:],
                        op0=ALU.mult, op1=ALU.add)

                # ---- batched dn transpose, then dp per pair ----
                etp = ps2.tile([96, 128], bf16, tag="tr")
                nc.tensor.transpose(etp[:], dna[:], idb)
                ets = cp.tile([96, 128], bf16, tag="ets")
                nc.vector.tensor_copy(ets[:], etp[:])
                for p in range(NP):
                    s3 = slice(32 * p, 32 * p + 16)
                    ddp = ps2.tile([128, 32], f32, tag="ddp")
                    nc.tensor.matmul(ddp[:], gzF[s3, :], wpk[s3, 32:64],
                                     start=True, stop=False)
                    nc.tensor.matmul(ddp[:], ets[s3, :], wpk[s3, 64:96],
                                     start=False, stop=True)
                    dpo = op3.tile([128, 32], f32, tag="dpo")
                    nc.vector.tensor_copy(dpo[:], ddp[:])
                    nc.sync.dma_start(dp_d[p], dpo[:])

    nc.finalize()
    return nc


def _prepare_in_maps(v, e, m, p, q, mvw, W_T, W1_w, W1_b, W_F):
    import ml_dtypes
    f32 = np.float32
    bf16 = ml_dtypes.bfloat16
    v, m, p, q, mvw = (np.asarray(x, f32) for x in (v, m, p, q, mvw))
    W_T, W1_w, W1_b, W_F = (np.asarray(x, f32) for x in (W_T, W1_w, W1_b, W_F))

    vs = (1.0 / (1.0 + np.exp(-v))).astype(f32)
    vq = np.concatenate([vs, q], axis=1)                      # [N, 96]
    R = (vq @ W1_w.T).astype(f32)                             # [N, 16]
    U = (R + W1_b[None, :]).astype(f32)
    rn2 = np.einsum("nh,nh->n", R, R).astype(f32)
    un2 = np.einsum("nh,nh->n", U, U).astype(f32)
    zT = (np.concatenate([vs, p], axis=1) @ W_T.T).astype(f32)
    zF = (p @ W_F.T).astype(f32)

    mol_id = np.argmax(mvw, axis=0)                           # [N]
    w_node = (mvw[mol_id, np.arange(N)] * m[:, 0]).astype(f32)

    sizes = np.bincount(mol_id, minlength=NM)
    assert sizes.max() <= SLOT, f"molecule of size {sizes.max()} > {SLOT}"
    order = np.argsort(-sizes, kind="stable")
    nodes_of = [np.where(mol_id == k)[0] for k in range(NM)]

    # weights replicated at partition bases 0/32/64 (matmul lhsT/rhs bases
    # must match); cols: [WTp | -WF | W1q]
    wkb = np.concatenate([W_T[:, VD:], -W_F, W1_w[:, VD:]], axis=1)
    wpk = np.zeros((80, 96), f32)
    for b0 in (0, 32, 64):
        wpk[b0:b0 + H, :] = wkb

    shared = {"wpk": np.ascontiguousarray(wpk.astype(bf16))}
    in_maps = []
    scatter = []    # per core: (dram_flat_row, node_idx) pairs
    for c in range(NCORES):
        mols = [order[i] for i in range(c, NM, NCORES)]
        su = np.zeros((108, 320), f32)
        rowpk = np.zeros((128, RW), f32)
        bfpk = np.zeros((128, BFW), bf16)
        bfpk[:, NP * 18:NP * 18 + 128] = np.eye(128, dtype=bf16)
        dm = np.ones((128, FW), bf16)
        for pp in range(NP):
            for t in range(SLOT):
                dm[t, 64 * pp + t] = 0
                dm[64 + t, 64 * pp + t] = 0
        bfpk[:, NP * 18 + 128:] = dm
        upk = np.zeros((SLOT, 2 * NP * 18), bf16)
        sc = []
        for b, k in enumerate(mols):
            idx = nodes_of[k]
            n = len(idx)
            pp, h = b // 2, b % 2
            r0 = 64 * h
            # S matmul: K-group rows 36*pp + 18*h
            k0 = 36 * pp + 18 * h
            su[k0:k0 + 16, r0:r0 + n] = -2.0 * R[idx].T
            su[k0 + 16, r0:r0 + n] = rn2[idx]
            su[k0 + 17, r0:r0 + n] = 1.0
            su[k0:k0 + 16, 128 + 64 * pp:128 + 64 * pp + n] = U[idx].T
            su[k0 + 16, 128 + 64 * pp:128 + 64 * pp + n] = 1.0
            su[k0 + 17, 128 + 64 * pp:128 + 64 * pp + n] = un2[idx]
            # row-wise packed data at rows r0:r0+n of pair tile pp
            wn = w_node[idx]
            rowpk[r0:r0 + n, 32 * pp:32 * pp + 16] = zT[idx]
            rowpk[r0:r0 + n, 32 * pp + 16:32 * pp + 32] = zF[idx]
            rowpk[r0:r0 + n, 96 + pp] = 2.0 / m[idx, 0]
            rowpk[r0:r0 + n, 99 + pp] = 2.0 * wn
            rowpk[r0:r0 + n, 102 + 16 * pp:118 + 16 * pp] = \
                -2.0 * wn[:, None] * U[idx]
            rowpk[r0:r0 + n, 150 + 16 * pp:166 + 16 * pp] = \
                -2.0 * wn[:, None] * R[idx]
            # P rhs [w r | w | 0] rows at partition r0..
            bfpk[r0:r0 + n, 18 * pp:18 * pp + 16] = wn[:, None] * R[idx]
            bfpk[r0:r0 + n, 18 * pp + 16] = wn
            # B rhs [w u | 0 | w] rows at partitions 0:n
            upk[0:n, 18 * b:18 * b + 16] = wn[:, None] * U[idx]
            upk[0:n, 18 * b + 17] = wn
            for t in range(n):
                sc.append((pp * 128 + r0 + t, idx[t]))
        in_maps.append({
            **shared,
            "su": np.ascontiguousarray(su),
            "rowpk": np.ascontiguousarray(rowpk),
            "bfpk": np.ascontiguousarray(bfpk),
            "upk": np.ascontiguousarray(upk),
        })
        scatter.append(sc)
    return in_maps, scatter


def _ensure_ntff_hook():
    """Make antenv.axon_hooks importable so bass_utils' trace path works."""
    try:
        from antenv.axon_hooks import get_axon_ntff_profile_hook  # noqa: F401
        return True
    except ImportError:
        pass
    import contextlib
    import ctypes
    import sys
    import types

    so_path = "/opt/axon/libaxon_pjrt.so"
    try:
        lib = ctypes.CDLL(so_path)
    except OSError:
        return False
    if not hasattr(lib, "axon_start_nrt_profile"):
        return False
    lib.axon_start_nrt_profile.argtypes = [
        ctypes.POINTER(ctypes.c_int64),
        ctypes.c_size_t,
    ]
    lib.axon_start_nrt_profile.restype = ctypes.c_int64
    lib.axon_stop_nrt_profile.argtypes = [ctypes.c_char_p]
    lib.axon_stop_nrt_profile.restype = ctypes.c_int64

    @contextlib.contextmanager
    def _hook(output_dir, device_ids):
        import jax

        jax.devices()
        if device_ids:
            ids = (ctypes.c_int64 * len(device_ids))(*device_ids)
            rc = lib.axon_start_nrt_profile(ids, len(device_ids))
        else:
            rc = lib.axon_start_nrt_profile(None, 0)
        if rc != 0:
            raise RuntimeError(f"axon_start_nrt_profile rc={rc}")
        try:
            yield
        finally:
            n = lib.axon_stop_nrt_profile(str(output_dir).encode())
            if n < 0:
                raise RuntimeError(f"axon_stop_nrt_profile rc={n}")

    mod = types.ModuleType("antenv.axon_hooks")
    mod.get_axon_ntff_profile_hook = lambda: _hook
    sys.modules["antenv.axon_hooks"] = mod
    try:
        import antenv

        antenv.axon_hooks = mod
    except ImportError:
        pass
    return True


def kernel(v, e, m, p, q, mvw, W_T, W1_w, W1_b, W_F):
    from concourse.bass_utils import run_bass_kernel_spmd

    in_maps, scatter = _prepare_in_maps(v, e, m, p, q, mvw,
                                        W_T, W1_w, W1_b, W_F)

    if "nc" not in _CACHE:
        _CACHE["nc"] = _build_nc()
    nc = _CACHE["nc"]

    trace = bool(os.environ.get("BASS_KERNEL_TRACE")) and _ensure_ntff_hook()
    res = run_bass_kernel_spmd(nc, in_maps, list(range(NCORES)), trace=trace)
    if trace and res.exec_time_ns is not None:
        print(f"HW exec time: {res.exec_time_ns} ns")

    dp = np.zeros((N, 32), np.float32)
    dq = np.zeros((N, 32), np.float32)
    for c in range(NCORES):
        dps = res.results[c]["dp_s"].reshape(NP * 128, 32)
        dqs = res.results[c]["dq_s"].reshape(NP * 128, 32)
        rows = np.array([r for r, _ in scatter[c]])
        nodes = np.array([nidx for _, nidx in scatter[c]])
        dp[nodes] = dps[rows]
        dq[nodes] = dqs[rows]
    return dp, dq
